# revision 29
# baseline (speedup 1.0000x reference)
"""Trainium2 Bass kernel for nn_LinkPredictor (MoE-routed bilinear link scorer).

score[b] = head[b]^T @ W[rel_id[b]] @ tail[b] + sum(b[rel_id[b]])

Strategy (relation sharding / MoE routing on host, dense matmuls on device):
  * Host groups samples by relation (argsort of rel_id), splits each
    relation's samples into slots of <=128, sorts slots by size and deals
    them round-robin-by-rank to the 8 NeuronCores so the static per-slot
    capacity cap_j (max over cores, 16-aligned) stays tight.  Cores
    process slots smallest-first.
  * Per slot the device computes Q = H_slot @ W[r] via 4 PE matmuls
    (contraction over e in chunks of 128; stationary = transposed heads
    [e_chunk, samples] fp16, moving = W[r][e_chunk, :] fp16), accumulated
    in one PSUM bank as Q[sample, d].
  * DVE: score = reduce_add(Q * tail, axis=free) in one fused
    affine_mul_reduce (accumulator seeds at zero - no memset needed).
    The relation bias column-sum is folded into the host-side gather.
  * W is only ever read once from HBM across the whole machine
    (16 MB fp16 total -> 2 MB per core), which is the bandwidth floor.
  * The PE-feeding stream rides the sync HWDGE ring in exact consumption
    order: per slot, [ht_j | W chunks 0-1] then [W chunks 2-3].  Tails
    (DVE-only) and the output ride the scalar ring.  Every DMA sources a
    dedicated C-contiguous dram tensor - a pure linear HBM sweep (strided
    row reads measurably cap the stream ~10% lower).
  * A burst of dummy matmuls issues right after the preamble, sized to end
    when the first slot's data lands, so the PE HAM clock-gate (cold
    1.2 GHz -> warm 2.4 GHz after ~3.4 us sustained activity) opens just
    as real matmuls begin.
  * Raw bacc (no TileContext): hand-placed semaphores, no buffer reuse.
"""

import os
import sys
import math

import numpy as np

for _p in ("/opt/trn_rl_repo",):
    if _p not in sys.path:
        sys.path.append(_p)

import concourse.bass as bass  # noqa: E402
import concourse.mybir as mybir  # noqa: E402
from concourse import bacc  # noqa: E402
from concourse import bass_utils  # noqa: E402

B, D, R = 2048, 512, 32
N_CORES = 8
F32 = mybir.dt.float32
# Matmul operand dtype for W, transposed heads, and tails.  float16 keeps 10
# mantissa bits (~3e-4 scale-relative absmax on this problem, vs the 2e-3
# gate) while halving the dominant weight-stream bytes.
MM_DT = mybir.dt.float16
MM_NP = np.float16
# PE warm-up: NWARM dummy matmuls (N=WARM_N) in one gapless burst.  The HAM
# clock-gate (1.2 -> 2.4 GHz) only opens after ~3.4 us of CONTINUOUS PE
# activity, and real matmuls chase the DMA stream with ~0.5 us waits that
# keep resetting the busy streak - so the warm-up burst alone must span the
# full ~3.4 us window, timed to end when the first slot's data lands.
NWARM = 8
WARM_N = 512


def _install_ntff_hook():
    """Provide antenv.axon_hooks if the image lacks it, so trace=True /
    BASS_TRACE=1 profiling works under axon (see trn_agent_boot.trn_boot)."""
    try:
        from antenv.axon_hooks import get_axon_ntff_profile_hook  # noqa: F401
        return
    except ImportError:
        pass
    import types
    try:
        import antenv
        from trn_agent_boot.trn_boot import _ntff_profile_via_ctypes
    except Exception:
        return
    mod = types.ModuleType("antenv.axon_hooks")
    _state = {"hook": None}
    try:
        _state["hook"] = _ntff_profile_via_ctypes("/opt/axon/libaxon_pjrt.so")
    except Exception:
        _state["hook"] = None

    def set_axon_ntff_profile_hook(h):
        _state["hook"] = h

    def get_axon_ntff_profile_hook():
        return _state["hook"]

    mod.set_axon_ntff_profile_hook = set_axon_ntff_profile_hook
    mod.get_axon_ntff_profile_hook = get_axon_ntff_profile_hook
    sys.modules["antenv.axon_hooks"] = mod
    antenv.axon_hooks = mod


_install_ntff_hook()

_PROGRAM_CACHE = {}


def _build_program(S, caps):
    """Raw-bacc program for one core: S slots, slot j holds cap_j samples of
    one relation.  caps is a tuple of per-slot capacities (<=128)."""
    caps = list(caps)
    M = sum(caps)
    offs = [0]
    for c_ in caps:
        offs.append(offs[-1] + c_)
    # Per-slot tile column layout: [ht (4*cap) | W (4*512)]
    tcols = [4 * c_ + 4 * D for c_ in caps]
    assert S + 1 <= 8, f"need {S + 1} PSUM banks"

    nc = bacc.Bacc("TRN2", target_bir_lowering=False, debug=False,
                   num_devices=N_CORES)

    # One dram tensor per DMA, each C-contiguous, so every DMA is a pure
    # linear HBM sweep (strided row reads measurably cap the stream ~10%
    # lower).  wa_j = [ht_j | W chunks 0-1], wb_j = [W chunks 2-3].
    wa = [nc.dram_tensor(f"wa{j}", [128, 4 * caps[j] + 2 * D], MM_DT,
                         kind="ExternalInput") for j in range(S)]
    wb = [nc.dram_tensor(f"wb{j}", [128, 2 * D], MM_DT,
                         kind="ExternalInput") for j in range(S)]
    # Tails in slot-PAIR groups [rows, 2, D]: 2KB-per-partition contiguous
    # descriptors without padding every slot to 128 rows (per-slot [cap, D]
    # DMAs have 1KB descriptors and measured ~5x lower queue throughput;
    # a single [128, S, D] pad costs +200KB of stream).
    tgroups = [(2 * g, min(2 * g + 1, S - 1)) for g in range((S + 1) // 2)]
    trows = [max(caps[j0:j1 + 1]) for j0, j1 in tgroups]
    tg = [nc.dram_tensor(f"tg{g}", [trows[g], (j1 - j0 + 1) * D], MM_DT,
                         kind="ExternalInput")
          for g, (j0, j1) in enumerate(tgroups)]
    out = nc.dram_tensor("out", [128, S], F32, kind="ExternalOutput")

    import contextlib
    with contextlib.ExitStack() as ctx:
        # no_gpsimd_drain: gpsimd has no instructions in this kernel, so the
        # block-end barrier can skip its dge_drain and use the cheaper
        # sem-only barrier.
        block = ctx.enter_context(nc.Block(no_gpsimd_drain=True))
        # One semaphore per DMA (a sem shared by several DMAs only supports
        # waits at the final total: per-engine completions interleave).
        sem_a = [ctx.enter_context(nc.semaphore(f"sem_a{j}"))
                 for j in range(S)]
        sem_b = [ctx.enter_context(nc.semaphore(f"sem_b{j}"))
                 for j in range(S)]
        sem_tl = [ctx.enter_context(nc.semaphore(f"sem_tl{g}"))
                  for g in range(len(tgroups))]
        # Last slot's W chunks 2/3 ride separate 128KB DMAs: the completion
        # semaphore of a big DMA lags its last data byte by the slowest of
        # 16 SDMA engines (~0.5-1 us measured); smaller final DMAs shrink it.
        sem_b3 = ctx.enter_context(nc.semaphore("sem_bx"))
        sem_mm = ctx.enter_context(nc.semaphore("sem_mm"))  # PE -> DVE
        sem_v = ctx.enter_context(nc.semaphore("sem_v"))    # DVE -> out DMA
        sem_o = ctx.enter_context(nc.semaphore("sem_o"))    # out DMA done

        wt_t = [ctx.enter_context(
            nc.sbuf_tensor(f"wt{j}", [128, tcols[j]], MM_DT))
            for j in range(S)]
        tlp_t = ctx.enter_context(
            nc.sbuf_tensor("tlps", [128, S, D], MM_DT))
        prod_t = ctx.enter_context(nc.sbuf_tensor("prod", [128, D], F32))
        score_t = ctx.enter_context(nc.sbuf_tensor("score", [128, S], F32))
        # Warm-up operands: never written; garbage contents are fine (the
        # warm-up PSUM bank is never read).
        dum_t = ctx.enter_context(nc.sbuf_tensor("dum", [128, WARM_N], MM_DT))
        psum_t = [ctx.enter_context(
            nc.psum_tensor(f"P{j}", [128, D], F32)) for j in range(S)]
        psum_w = ctx.enter_context(nc.psum_tensor("Pw", [128, D], F32))

        @block.sync
        def _(sync):
            # Slot tiles in consumption order on the sync ring.  Adding more
            # DMAs to this queue serializes their completion-receipt stalls
            # (measured +1.5 us sem lag with 9 queued), so tails ride the
            # scalar ring instead.
            for j in range(S):
                h1 = 4 * caps[j] + 2 * D  # ht + W chunks 0-1
                sync.dma_start(
                    wt_t[j].ap()[:, :h1], wa[j].ap()).then_inc(sem_a[j], 16)
                if j < S - 1:
                    sync.dma_start(
                        wt_t[j].ap()[:, h1:], wb[j].ap()).then_inc(
                        sem_b[j], 16)
                else:
                    sync.dma_start(
                        wt_t[j].ap()[:, h1:h1 + D],
                        wb[j].ap()[:, :D]).then_inc(sem_b[j], 16)
                    sync.dma_start(
                        wt_t[j].ap()[:, h1 + D:],
                        wb[j].ap()[:, D:]).then_inc(sem_b3, 16)

        @block.scalar
        def _(scalar):
            # Tails ride the scalar ring; they contend with the tile stream
            # only briefly (327KB vs the 2.4MB tile stream).
            for g, (j0, j1) in enumerate(tgroups):
                scalar.dma_start(
                    tlp_t.ap()[:trows[g], j0:j1 + 1, :],
                    tg[g].ap().rearrange("p (s d) -> p s d", s=j1 - j0 + 1),
                ).then_inc(sem_tl[g], 16)
            # Output store, gated on the final DVE reduce.  No completion
            # wait: the data lands mid-postamble, ~5 us before program end
            # (the walrus postamble serially resets the whole semaphore
            # space), and nothing reads sem_o - waiting would only push the
            # postamble later.
            scalar.wait_ge(sem_v, 1)
            scalar.dma_start(out.ap(), score_t.ap()).then_inc(sem_o, 16)

        @block.tensor
        def _(tensor):
            # HAM warm-up: keep the PE busy from the end of the preamble
            # until the first tile lands, so real matmuls run at 2.4 GHz.
            for _i in range(NWARM):
                tensor.matmul(
                    psum_w.ap()[:, :WARM_N],
                    dum_t.ap()[:, :128],
                    dum_t.ap(),
                    start=True, stop=True,
                )
            for j in range(S):
                cap = caps[j]
                for c in range(4):
                    if c % 2 == 0:
                        tensor.wait_ge((sem_a[j], sem_b[j])[c // 2], 16)
                    if c == 3 and j == S - 1:
                        tensor.wait_ge(sem_b3, 16)
                    mm = tensor.matmul(
                        psum_t[j].ap()[:cap, :],
                        wt_t[j].ap()[:, c * cap:(c + 1) * cap],
                        wt_t[j].ap()[:, 4 * cap + c * D:4 * cap + (c + 1) * D],
                        start=(c == 0),
                        stop=(c == 3),
                    )
                mm.then_inc(sem_mm, 1)

        @block.vector
        def _(vector):
            for j in range(S):
                vector.wait_ge(sem_tl[j // 2], 16)
                vector.wait_ge(sem_mm, j + 1)
                # fused (Q * tail) + row-sum in one DVE op; accum seeds at 0
                r = vector.affine_mul_reduce(
                    out=prod_t.ap()[:caps[j], :],
                    accum_out=score_t.ap()[:caps[j], j:j + 1],
                    in0=psum_t[j].ap()[:caps[j], :],
                    in1=tlp_t.ap()[:caps[j], j, :],
                    scale=1.0, bias=0.0,
                )
            r.then_inc(sem_v, 1)

    nc.compile()
    return nc


def _route(rel):
    """Group samples by relation into slots of <=128; deal round-robin by
    rank so per-slot capacities stay tight.

    Returns (S, caps, core_slots): core_slots[c] is a list of exactly S
    (relation, sample_indices) pairs, sorted by size ASC (smallest slot
    first so the PE starts earliest); caps[j] is the static capacity of
    slot j (max over cores, 16-aligned)."""
    counts = np.bincount(rel, minlength=R)
    order = np.argsort(rel, kind="stable")
    slots = []
    off = 0
    for r in range(R):
        n = int(counts[r])
        idx = order[off:off + n]
        off += n
        for c0 in range(0, n, 128):
            slots.append((r, idx[c0:c0 + 128]))
    slots.sort(key=lambda s: len(s[1]))  # ascending
    S = max(1, math.ceil(len(slots) / N_CORES))
    empty = np.zeros(0, dtype=np.int64)
    while len(slots) < S * N_CORES:
        slots.insert(0, (0, empty))
    # core c slot j = rank j*8+c: slot-j sizes are adjacent in sorted order,
    # so cap_j = max over the group is tight.
    core_slots = [[slots[j * N_CORES + c] for j in range(S)]
                  for c in range(N_CORES)]
    caps = tuple(
        min(128, max(16, 16 * math.ceil(
            max(len(core_slots[c][j][1]) for c in range(N_CORES)) / 16)))
        for j in range(S))
    return S, caps, core_slots


def _marshal(head_emb, tail_emb, rel, W, b):
    """Route + build per-core input maps (device-ready layouts)."""
    S, caps, core_slots = _route(rel)
    bsum = b.astype(np.float64).sum(axis=1).astype(np.float32)
    Wh = W.astype(MM_NP)
    # Wr[r]: [128, 4, D] with Wr[r][p, cc, d] = W[r][cc*128+p, d]
    Wr = np.ascontiguousarray(
        Wh.reshape(R, 4, 128, D).transpose(0, 2, 1, 3))

    tgroups = [(2 * g, min(2 * g + 1, S - 1)) for g in range((S + 1) // 2)]
    trows = [max(caps[j0:j1 + 1]) for j0, j1 in tgroups]

    in_maps = []
    for c in range(N_CORES):
        im = {}
        tgm = [np.zeros((trows[g], j1 - j0 + 1, D), dtype=MM_NP)
               for g, (j0, j1) in enumerate(tgroups)]
        for j, (r, idx) in enumerate(core_slots[c]):
            n = len(idx)
            cap = caps[j]
            wam = np.zeros((128, 4 * cap + 2 * D), dtype=MM_NP)
            if n:
                # ht part: wam[p, cc*cap + k] = head_emb[idx_k, cc*128+p]
                wam[:, :4 * cap].reshape(128, 4, cap)[:, :, :n] = (
                    head_emb[idx].T.reshape(4, 128, n).transpose(1, 0, 2))
                tgm[j // 2][:n, j - tgroups[j // 2][0], :] = tail_emb[idx]
            wam[:, 4 * cap:] = Wr[r][:, :2, :].reshape(128, 2 * D)
            im[f"wa{j}"] = wam
            im[f"wb{j}"] = np.ascontiguousarray(
                Wr[r][:, 2:, :].reshape(128, 2 * D))
        for g, (j0, j1) in enumerate(tgroups):
            im[f"tg{g}"] = tgm[g].reshape(trows[g], (j1 - j0 + 1) * D)
        in_maps.append(im)
    return S, caps, core_slots, in_maps, bsum


def kernel(head_emb, tail_emb, rel_id, W, b, **_unused):
    head_emb = np.ascontiguousarray(np.asarray(head_emb, dtype=np.float32))
    tail_emb = np.ascontiguousarray(np.asarray(tail_emb, dtype=np.float32))
    W = np.ascontiguousarray(np.asarray(W, dtype=np.float32))
    b = np.ascontiguousarray(np.asarray(b, dtype=np.float32))
    rel = np.asarray(rel_id).astype(np.int64)

    S, caps, core_slots, in_maps, bsum = _marshal(
        head_emb, tail_emb, rel, W, b)

    key = (S, caps)
    if key not in _PROGRAM_CACHE:
        _PROGRAM_CACHE[key] = _build_program(S, caps)
    nc = _PROGRAM_CACHE[key]

    res = bass_utils.run_bass_kernel_spmd(nc, in_maps,
                                          core_ids=list(range(N_CORES)))

    scores = np.zeros(head_emb.shape[0], dtype=np.float32)
    for c in range(N_CORES):
        o = res.results[c]["out"]
        for j, (r, idx) in enumerate(core_slots[c]):
            n = len(idx)
            if n:
                scores[idx] = o[:n, j] + bsum[r]
    return scores
